# revision 10
# baseline (speedup 1.0000x reference)
"""Single-head full attention (B=4, S=4096, D=512) on 8 TRN2 NeuronCores.

Sharding: core c handles batch b = c//2, query half h = c%2 (2048 queries).

Key algebraic fold: scores = (x_q Wq^T)(x Wk^T)^T / sqrt(D)
                           = x_q @ M @ x^T,   M = Wq^T Wk / sqrt(D)  (host).
So K is never materialized: x^T itself (resident in SBUF, fp16) is the
stationary operand of the scores matmul, and T = x_q @ M replaces Q.
Per-query additive terms drop out of softmax (row-shift invariance); with
biases the per-key additive beta[j] = (bq Wk/sqrt(D))x[j]^T is applied as a
multiplier exp(beta) on the exp'd scores (the bq.bk constant cancels).

Each core's xT is pre-ROTATED on host so its own query half occupies
columns 0..2047; the T projection then just slices xt_sb, no separate xq
tensor (softmax and PV sum over keys in any fixed order, and V is computed
in the same rotated order, so the output is unchanged).

Device layouts (per core, fp16 operands, fp32 accumulate):
  xt_sb [128, 4, 4096]: x^T rotated, partition p + tile t -> d' = t*128+p
  tt_sb [128, 4, 2048]: T^T = (x_q @ M)^T
  v_sb  [128, 32, 512]: V natural, partition p + block jb -> j = jb*128+p
Scores are computed transposed (S^T[j, q]) so exp(S^T) blocks serve directly
as the stationary operand of the P@V matmul, producing O in natural [q, d]
orientation. exp runs in two [128,256] chunks so the first PV matmul only
waits on a half-tile (ACT latency < the covering scores matmuls). Softmax
denominators come from an N=2 ones-matmul sharing the same stationary tile;
the four per-subblock denominator groups share one PSUM bank, zeroed once,
all groups accumulating with start=False. No max-subtraction: scores are
O(1) and softmax is shift-invariant. Output y DMAs go out on the
Activation HWDGE queue so they never serialize against the next
iteration's input loads on the SP queue.
"""
import math
import numpy as np

B, S, D = 4, 4096, 512
P = 128
SQ = S // 2          # queries per core
NCORES = 8
QTILE = 512          # query columns per score/PV pass

last_results = None  # BassKernelResults of the most recent run (for test.py)

_nc_cache = {}


def _build_nc(has_bias, has_mask, reps=1):
    import concourse.bacc as bacc
    import concourse.tile as tile
    from concourse import mybir
    from contextlib import ExitStack

    f32 = mybir.dt.float32
    f16 = mybir.dt.float16
    Exp = mybir.ActivationFunctionType.Exp

    nc = bacc.Bacc("TRN2", target_bir_lowering=False, debug=False)
    xT = nc.declare_dram_parameter("xT", [D, S], f16, False)
    mT = nc.declare_dram_parameter("mT", [D, D], f16, False)
    wvT = nc.declare_dram_parameter("wvT", [D, D], f16, False)
    if has_bias:
        wtl = nc.declare_dram_parameter("wtl", [P, D // P], f16, False)
        bvr = nc.declare_dram_parameter("bvr", [P, D], f32, False)
    if has_mask:
        maskf = nc.declare_dram_parameter("maskf", [P, S // P], f32, False)
    y = nc.declare_dram_parameter("y", [SQ, D], f32, True)

    ET = D // P          # 4 d'-tiles
    NJB = S // P         # 32 key blocks
    NQT = SQ // QTILE    # 4 query tiles
    NQS = QTILE // P     # 4 query subblocks per tile
    EC = 2               # exp chunks per score tile
    ECW = QTILE // EC    # exp chunk width (query cols)

    with tile.TileContext(nc) as tc, ExitStack() as ctx:
        wpool = ctx.enter_context(tc.tile_pool(name="wpool", bufs=1))
        # xt is double-buffered so unrolled bench bodies can prefetch the
        # next iteration's x^T while the current one still computes
        bigx = ctx.enter_context(tc.tile_pool(name="bigx", bufs=2))
        big = ctx.enter_context(tc.tile_pool(name="big", bufs=1))
        expp = ctx.enter_context(tc.tile_pool(name="expp", bufs=6))
        outp = ctx.enter_context(tc.tile_pool(name="outp", bufs=4))
        smallp = ctx.enter_context(tc.tile_pool(name="smallp", bufs=3))
        # PSUM: shared [128,512] accumulate tag (projections + scores) keeps
        # every phase inside 8 banks: 3 (mm512) + 4 (po) + 1 (sums).
        psum_mm = ctx.enter_context(tc.tile_pool(name="psum_mm", bufs=3, space="PSUM"))
        psum_o = ctx.enter_context(tc.tile_pool(name="psum_o", bufs=1, space="PSUM"))
        psum_sum = ctx.enter_context(tc.tile_pool(name="psum_sum", bufs=1, space="PSUM"))

        m_sb = wpool.tile([P, ET, D], f16)
        wv_sb = wpool.tile([P, ET, D], f16)
        # weights ride the ACT HWDGE queue so they land in parallel with the
        # first xT chunks on the SP queue (single-shot ramp)
        nc.scalar.dma_start(out=m_sb, in_=mT[:, :].rearrange("(t p) e -> p t e", p=P))
        nc.scalar.dma_start(out=wv_sb, in_=wvT[:, :].rearrange("(t p) e -> p t e", p=P))
        ones_sb = wpool.tile([P, 2], f16)
        nc.vector.memset(ones_sb, 1.0)
        if has_bias:
            wtl_sb = wpool.tile([P, D // P], f16)
            bv_sb = wpool.tile([P, D], f32)
            nc.sync.dma_start(out=wtl_sb, in_=wtl[:, :])
            nc.sync.dma_start(out=bv_sb, in_=bvr[:, :])
        if has_mask:
            mask_sb = wpool.tile([P, S // P], f32)
            nc.sync.dma_start(out=mask_sb, in_=maskf[:, :])

        tt_sb = big.tile([P, ET, SQ], f16)
        v_sb = big.tile([P, NJB, D], f16)

        xT_r = xT[:, :].rearrange("(t p) s -> p t s", p=P)

        def body(rep):
            xt_sb = bigx.tile([P, ET, S], f16, tag="xt", name=f"xt_{rep}")
            # x^T chunks in natural order: the rotated layout puts this
            # core's query half (T-proj input) in the first SQ columns
            for c in range(S // QTILE):
                nc.sync.dma_start(
                    out=xt_sb[:, :, c * QTILE:(c + 1) * QTILE],
                    in_=xT_r[:, :, c * QTILE:(c + 1) * QTILE])

            # T^T projection: M-stationary, x_q^T-moving (query half of xt)
            for c in range(SQ // QTILE):
                for me in range(ET):
                    pq = psum_mm.tile([P, QTILE], f32, tag="mm512",
                                      name=f"pq_{rep}_{c}_{me}")
                    for t in range(ET):
                        nc.tensor.matmul(
                            pq,
                            lhsT=m_sb[:, t, me * P:(me + 1) * P],
                            rhs=xt_sb[:, t, c * QTILE:(c + 1) * QTILE],
                            start=(t == 0), stop=(t == ET - 1))
                    nc.scalar.copy(out=tt_sb[:, me, c * QTILE:(c + 1) * QTILE], in_=pq)

            # V projection: x^T-stationary, Wv^T-moving
            for sb_i in range(NJB):
                pv = psum_mm.tile([P, D], f32, tag="mm512", name=f"pv_{rep}_{sb_i}")
                for t in range(ET):
                    nc.tensor.matmul(
                        pv,
                        lhsT=xt_sb[:, t, sb_i * P:(sb_i + 1) * P],
                        rhs=wv_sb[:, t, :],
                        start=(t == 0), stop=(t == ET - 1))
                nc.vector.tensor_copy(out=v_sb[:, sb_i, :], in_=pv)

            # per-key bias multiplier exp(beta[j]) (only when biases present)
            if has_bias:
                bmul_sb = smallp.tile([P, NJB], f32, tag="bmul", name=f"bm_{rep}")
                for jb in range(NJB):
                    pb = psum_sum.tile([P, 2], f32, tag="bsum", name=f"pb_{rep}_{jb}")
                    for t in range(ET):
                        nc.tensor.matmul(
                            pb,
                            lhsT=xt_sb[:, t, jb * P:(jb + 1) * P],
                            rhs=wtl_sb[:, t:t + 1].to_broadcast([P, 2]),
                            start=(t == 0), stop=(t == ET - 1))
                    nc.scalar.activation(out=bmul_sb[:, jb:jb + 1], in_=pb[:, 0:1],
                                         func=Exp, scale=1.0)

            # attention — software-pipelined so the PE program order carries
            # two score tiles of cover ahead of each PV group: at qt
            # boundaries PV(qt+1, jb0) must wait for the DVE normalization
            # of qt to release the po banks (~1.4us), which two in-flight
            # score groups (~1.7us) fully hide.
            for qt in range(NQT):
                po = [psum_o.tile([P, D], f32, tag=f"po{qs}", name=f"po_{rep}_{qt}_{qs}")
                      for qs in range(NQS)]
                psums = psum_sum.tile([P, 2 * NQS], f32, tag="sums",
                                      name=f"sums_{rep}_{qt}")
                nc.vector.memset(psums, 0.0)
                pexps = {}

                def scores(jb):
                    ps_t = psum_mm.tile([P, QTILE], f32, tag="mm512",
                                        name=f"ps_{rep}_{qt}_{jb}")
                    for t in range(ET):
                        nc.tensor.matmul(
                            ps_t,
                            lhsT=xt_sb[:, t, jb * P:(jb + 1) * P],
                            rhs=tt_sb[:, t, qt * QTILE:(qt + 1) * QTILE],
                            start=(t == 0), stop=(t == ET - 1))
                    pexp = expp.tile([P, QTILE], f16, tag="pexp",
                                     name=f"pe_{rep}_{qt}_{jb}")
                    # exp in EC chunks: PV(qs) only waits on its own chunk
                    for e in range(EC):
                        nc.scalar.activation(
                            out=pexp[:, e * ECW:(e + 1) * ECW],
                            in_=ps_t[:, e * ECW:(e + 1) * ECW],
                            func=Exp, scale=1.0)
                        if has_bias:
                            nc.vector.tensor_scalar_mul(
                                pexp[:, e * ECW:(e + 1) * ECW],
                                pexp[:, e * ECW:(e + 1) * ECW],
                                bmul_sb[:, jb:jb + 1])
                        if has_mask:
                            nc.vector.tensor_scalar_mul(
                                pexp[:, e * ECW:(e + 1) * ECW],
                                pexp[:, e * ECW:(e + 1) * ECW],
                                mask_sb[:, jb:jb + 1])
                    pexps[jb] = pexp

                def pv(jb):
                    pexp = pexps.pop(jb)
                    for qs in range(NQS):
                        nc.tensor.matmul(
                            po[qs],
                            lhsT=pexp[:, qs * P:(qs + 1) * P],
                            rhs=v_sb[:, jb, :],
                            start=(jb == 0), stop=(jb == NJB - 1))
                        nc.tensor.matmul(
                            psums[:, 2 * qs:2 * qs + 2],
                            lhsT=pexp[:, qs * P:(qs + 1) * P],
                            rhs=ones_sb,
                            start=False, stop=(jb == NJB - 1),
                            skip_group_check=True)

                # PV trails scores by 3 jb: each pv(jb) has ~3 score groups
                # (~2.6us) of PE work between exp(jb) issue and its first
                # consumer, absorbing HW ACT/sem latency the cost model
                # may underestimate (alive tiles: 3 mm512 bufs, 4 pexp)
                scores(0)
                scores(1)
                scores(2)
                for jb in range(NJB):
                    pv(jb)
                    if jb + 3 < NJB:
                        scores(jb + 3)
                recip = smallp.tile([P, 2 * NQS], f32, tag="recip", name=f"rc_{rep}_{qt}")
                nc.vector.reciprocal(out=recip, in_=psums)
                for qs in range(NQS):
                    o_sb = outp.tile([P, D], f32, tag="osb", name=f"o_{rep}_{qt}_{qs}")
                    nc.vector.tensor_scalar_mul(o_sb, po[qs], recip[:, 2 * qs:2 * qs + 1])
                    if has_bias:
                        nc.vector.tensor_add(out=o_sb, in0=o_sb, in1=bv_sb)
                    r0 = (qt * NQS + qs) * P
                    nc.scalar.dma_start(out=y[r0:r0 + P, :], in_=o_sb)

        if reps == 1:
            body(0)
        else:
            # bench-only loop. For_i carries an all-engine barrier + sem
            # reset at every back edge, so unroll U bodies per iteration:
            # the inner bodies pipeline freely (xt double-buffered) and the
            # barrier cost is paid once per U reps. Hint every engine so the
            # back-edge branch prefetches its IRAM block.
            U = 4
            assert reps % U == 0, reps
            with tc.For_i(0, reps // U, 1,
                          hint_engines=tuple(mybir.ALL_ENGINES)):
                for u in range(U):
                    body(u)
    nc.compile()
    return nc


def _prepare(x, mask, Wq, bq, Wk, bk, Wv, bv):
    """Build (or fetch cached) device program + per-core input maps."""
    x = np.asarray(x, dtype=np.float32)
    mask = np.asarray(mask)
    Wq = np.asarray(Wq, dtype=np.float32)
    Wk = np.asarray(Wk, dtype=np.float32)
    Wv = np.asarray(Wv, dtype=np.float32)
    bq = np.asarray(bq, dtype=np.float32)
    bk = np.asarray(bk, dtype=np.float32)
    bv = np.asarray(bv, dtype=np.float32)
    has_bias = bool(np.any(bq) or np.any(bk) or np.any(bv))
    has_mask = bool(np.any(mask))

    key = (has_bias, has_mask)
    if key not in _nc_cache:
        _nc_cache[key] = _build_nc(has_bias, has_mask)
    nc = _nc_cache[key]

    inv_sqrt_d = 1.0 / math.sqrt(D)
    M = (Wq.T.astype(np.float64) @ Wk.astype(np.float64)) * inv_sqrt_d
    mT_h = np.ascontiguousarray(M.astype(np.float32).astype(np.float16))
    wvT_h = np.ascontiguousarray(Wv.T.astype(np.float16))

    in_maps = []
    for c in range(NCORES):
        b, h = divmod(c, 2)
        xT_b = x[b].T.astype(np.float16)
        # rotate keys so this core's query half is columns 0..SQ-1
        xT_rot = np.ascontiguousarray(
            np.concatenate([xT_b[:, h * SQ:], xT_b[:, :h * SQ]], axis=1))
        m = {"xT": xT_rot, "mT": mT_h, "wvT": wvT_h}
        if has_bias:
            # per-key additive beta[j] = (bq Wk/sqrt(D)).x[j]; the bq.bk
            # constant shifts all keys equally and cancels in softmax.
            wt = (bq @ Wk) * inv_sqrt_d              # [D]
            m["wtl"] = np.ascontiguousarray(
                wt.reshape(D // P, P).T.astype(np.float16))
            m["bvr"] = np.ascontiguousarray(np.broadcast_to(bv, (P, D))).copy()
        if has_mask:
            keep = 1.0 - mask[b].astype(np.float32)
            keep_rot = np.concatenate([keep[h * SQ:], keep[:h * SQ]])
            m["maskf"] = np.ascontiguousarray(keep_rot.reshape(S // P, P).T)
        in_maps.append(m)
    return nc, in_maps


def _gather(res):
    out = np.empty((B, S, D), dtype=np.float32)
    for c in range(NCORES):
        b, h = divmod(c, 2)
        out[b, h * SQ:(h + 1) * SQ, :] = res.results[c]["y"]
    return out


def kernel(x, mask, Wq, bq, Wk, bk, Wv, bv):
    global last_results
    from concourse.bass_utils import run_bass_kernel_spmd

    nc, in_maps = _prepare(x, mask, Wq, bq, Wk, bk, Wv, bv)
    res = run_bass_kernel_spmd(nc, in_maps, core_ids=list(range(NCORES)))
    last_results = res
    return _gather(res)


# revision 11
# speedup vs baseline: 1.1795x; 1.1795x over previous
"""Single-head full attention (B=4, S=4096, D=512) on 8 TRN2 NeuronCores.

Sharding: core c handles batch b = c//2, query half h = c%2 (2048 queries).

Key algebraic fold: scores = (x_q Wq^T)(x Wk^T)^T / sqrt(D)
                           = x_q @ M @ x^T,   M = Wq^T Wk / sqrt(D)  (host).
So K is never materialized: x^T itself (resident in SBUF, fp16) is the
stationary operand of the scores matmul, and T = x_q @ M replaces Q.
Per-query additive terms drop out of softmax (row-shift invariance); with
biases the per-key additive beta[j] = (bq Wk/sqrt(D))x[j]^T is applied as a
multiplier exp(beta) on the exp'd scores (the bq.bk constant cancels).

Each core's xT is pre-ROTATED on host so its own query half occupies
columns 0..2047; the T projection then just slices xt_sb, no separate xq
tensor (softmax and PV sum over keys in any fixed order, and V is computed
in the same rotated order, so the output is unchanged).

Device layouts (per core, fp16 operands, fp32 accumulate):
  xt_sb [128, 4, 4096]: x^T rotated, partition p + tile t -> d' = t*128+p
  tt_sb [128, 4, 2048]: T^T = (x_q @ M)^T
  v_sb  [128, 32, 512]: V natural, partition p + block jb -> j = jb*128+p
Scores are computed transposed (S^T[j, q]) so exp(S^T) blocks serve directly
as the stationary operand of the P@V matmul, producing O in natural [q, d]
orientation. exp runs in two [128,256] chunks so the first PV matmul only
waits on a half-tile (ACT latency < the covering scores matmuls). Softmax
denominators come from an N=2 ones-matmul sharing the same stationary tile;
the four per-subblock denominator groups share one PSUM bank, zeroed once,
all groups accumulating with start=False. No max-subtraction: scores are
O(1) and softmax is shift-invariant. Output y DMAs go out on the
Activation HWDGE queue so they never serialize against the next
iteration's input loads on the SP queue.
"""
import math
import numpy as np

B, S, D = 4, 4096, 512
P = 128
SQ = S // 2          # queries per core
NCORES = 8
QTILE = 512          # query columns per score/PV pass

last_results = None  # BassKernelResults of the most recent run (for test.py)

_nc_cache = {}


def _build_nc(has_bias, has_mask, reps=1):
    import concourse.bacc as bacc
    import concourse.tile as tile
    from concourse import mybir
    from contextlib import ExitStack

    f32 = mybir.dt.float32
    f16 = mybir.dt.float16
    Exp = mybir.ActivationFunctionType.Exp

    nc = bacc.Bacc("TRN2", target_bir_lowering=False, debug=False)
    xT = nc.declare_dram_parameter("xT", [D, S], f16, False)
    mT = nc.declare_dram_parameter("mT", [D, D], f16, False)
    wvT = nc.declare_dram_parameter("wvT", [D, D], f16, False)
    if has_bias:
        wtl = nc.declare_dram_parameter("wtl", [P, D // P], f16, False)
        bvr = nc.declare_dram_parameter("bvr", [P, D], f32, False)
    if has_mask:
        maskf = nc.declare_dram_parameter("maskf", [P, S // P], f32, False)
    y = nc.declare_dram_parameter("y", [SQ, D], f32, True)

    ET = D // P          # 4 d'-tiles
    NJB = S // P         # 32 key blocks
    NQT = SQ // QTILE    # 4 query tiles
    NQS = QTILE // P     # 4 query subblocks per tile
    EC = 2               # exp chunks per score tile
    ECW = QTILE // EC    # exp chunk width (query cols)

    with tile.TileContext(nc) as tc, ExitStack() as ctx:
        wpool = ctx.enter_context(tc.tile_pool(name="wpool", bufs=1))
        # xt is double-buffered so unrolled bench bodies can prefetch the
        # next iteration's x^T while the current one still computes
        bigx = ctx.enter_context(tc.tile_pool(name="bigx", bufs=2))
        big = ctx.enter_context(tc.tile_pool(name="big", bufs=1))
        expp = ctx.enter_context(tc.tile_pool(name="expp", bufs=6))
        outp = ctx.enter_context(tc.tile_pool(name="outp", bufs=4))
        smallp = ctx.enter_context(tc.tile_pool(name="smallp", bufs=3))
        # PSUM: shared [128,512] accumulate tag (projections + scores) keeps
        # every phase inside 8 banks: 3 (mm512) + 4 (po) + 1 (sums).
        psum_mm = ctx.enter_context(tc.tile_pool(name="psum_mm", bufs=3, space="PSUM"))
        psum_o = ctx.enter_context(tc.tile_pool(name="psum_o", bufs=1, space="PSUM"))
        psum_sum = ctx.enter_context(tc.tile_pool(name="psum_sum", bufs=1, space="PSUM"))

        m_sb = wpool.tile([P, ET, D], f16)
        wv_sb = wpool.tile([P, ET, D], f16)
        # weights ride the ACT HWDGE queue so they land in parallel with the
        # first xT chunks on the SP queue (single-shot ramp)
        nc.scalar.dma_start(out=m_sb, in_=mT[:, :].rearrange("(t p) e -> p t e", p=P))
        nc.scalar.dma_start(out=wv_sb, in_=wvT[:, :].rearrange("(t p) e -> p t e", p=P))
        ones_sb = wpool.tile([P, 2], f16)
        nc.vector.memset(ones_sb, 1.0)
        if has_bias:
            wtl_sb = wpool.tile([P, D // P], f16)
            bv_sb = wpool.tile([P, D], f32)
            nc.sync.dma_start(out=wtl_sb, in_=wtl[:, :])
            nc.sync.dma_start(out=bv_sb, in_=bvr[:, :])
        if has_mask:
            mask_sb = wpool.tile([P, S // P], f32)
            nc.sync.dma_start(out=mask_sb, in_=maskf[:, :])

        tt_sb = big.tile([P, ET, SQ], f16)
        v_sb = big.tile([P, NJB, D], f16)

        xT_r = xT[:, :].rearrange("(t p) s -> p t s", p=P)

        def body(rep):
            xt_sb = bigx.tile([P, ET, S], f16, tag="xt", name=f"xt_{rep}")
            # x^T chunks in natural order: the rotated layout puts this
            # core's query half (T-proj input) in the first SQ columns
            for c in range(S // QTILE):
                nc.sync.dma_start(
                    out=xt_sb[:, :, c * QTILE:(c + 1) * QTILE],
                    in_=xT_r[:, :, c * QTILE:(c + 1) * QTILE])

            # T^T projection: M-stationary, x_q^T-moving (query half of xt)
            for c in range(SQ // QTILE):
                for me in range(ET):
                    pq = psum_mm.tile([P, QTILE], f32, tag="mm512",
                                      name=f"pq_{rep}_{c}_{me}")
                    for t in range(ET):
                        nc.tensor.matmul(
                            pq,
                            lhsT=m_sb[:, t, me * P:(me + 1) * P],
                            rhs=xt_sb[:, t, c * QTILE:(c + 1) * QTILE],
                            start=(t == 0), stop=(t == ET - 1))
                    nc.scalar.copy(out=tt_sb[:, me, c * QTILE:(c + 1) * QTILE], in_=pq)

            # V projection: x^T-stationary, Wv^T-moving
            for sb_i in range(NJB):
                pv = psum_mm.tile([P, D], f32, tag="mm512", name=f"pv_{rep}_{sb_i}")
                for t in range(ET):
                    nc.tensor.matmul(
                        pv,
                        lhsT=xt_sb[:, t, sb_i * P:(sb_i + 1) * P],
                        rhs=wv_sb[:, t, :],
                        start=(t == 0), stop=(t == ET - 1))
                nc.vector.tensor_copy(out=v_sb[:, sb_i, :], in_=pv)

            # per-key bias multiplier exp(beta[j]) (only when biases present)
            if has_bias:
                bmul_sb = smallp.tile([P, NJB], f32, tag="bmul", name=f"bm_{rep}")
                for jb in range(NJB):
                    pb = psum_sum.tile([P, 2], f32, tag="bsum", name=f"pb_{rep}_{jb}")
                    for t in range(ET):
                        nc.tensor.matmul(
                            pb,
                            lhsT=xt_sb[:, t, jb * P:(jb + 1) * P],
                            rhs=wtl_sb[:, t:t + 1].to_broadcast([P, 2]),
                            start=(t == 0), stop=(t == ET - 1))
                    nc.scalar.activation(out=bmul_sb[:, jb:jb + 1], in_=pb[:, 0:1],
                                         func=Exp, scale=1.0)

            # attention — software-pipelined so the PE program order carries
            # two score tiles of cover ahead of each PV group: at qt
            # boundaries PV(qt+1, jb0) must wait for the DVE normalization
            # of qt to release the po banks (~1.4us), which two in-flight
            # score groups (~1.7us) fully hide.
            for qt in range(NQT):
                po = [psum_o.tile([P, D], f32, tag=f"po{qs}", name=f"po_{rep}_{qt}_{qs}")
                      for qs in range(NQS)]
                psums = psum_sum.tile([P, 2 * NQS], f32, tag="sums",
                                      name=f"sums_{rep}_{qt}")
                nc.vector.memset(psums, 0.0)
                pexps = {}

                def scores(jb):
                    ps_t = psum_mm.tile([P, QTILE], f32, tag="mm512",
                                        name=f"ps_{rep}_{qt}_{jb}")
                    for t in range(ET):
                        nc.tensor.matmul(
                            ps_t,
                            lhsT=xt_sb[:, t, jb * P:(jb + 1) * P],
                            rhs=tt_sb[:, t, qt * QTILE:(qt + 1) * QTILE],
                            start=(t == 0), stop=(t == ET - 1))
                    pexp = expp.tile([P, QTILE], f16, tag="pexp",
                                     name=f"pe_{rep}_{qt}_{jb}")
                    # exp in EC chunks: PV(qs) only waits on its own chunk
                    for e in range(EC):
                        nc.scalar.activation(
                            out=pexp[:, e * ECW:(e + 1) * ECW],
                            in_=ps_t[:, e * ECW:(e + 1) * ECW],
                            func=Exp, scale=1.0)
                        if has_bias:
                            nc.vector.tensor_scalar_mul(
                                pexp[:, e * ECW:(e + 1) * ECW],
                                pexp[:, e * ECW:(e + 1) * ECW],
                                bmul_sb[:, jb:jb + 1])
                        if has_mask:
                            nc.vector.tensor_scalar_mul(
                                pexp[:, e * ECW:(e + 1) * ECW],
                                pexp[:, e * ECW:(e + 1) * ECW],
                                mask_sb[:, jb:jb + 1])
                    pexps[jb] = pexp

                def pv(jb):
                    pexp = pexps.pop(jb)
                    for qs in range(NQS):
                        nc.tensor.matmul(
                            po[qs],
                            lhsT=pexp[:, qs * P:(qs + 1) * P],
                            rhs=v_sb[:, jb, :],
                            start=(jb == 0), stop=(jb == NJB - 1))
                        nc.tensor.matmul(
                            psums[:, 2 * qs:2 * qs + 2],
                            lhsT=pexp[:, qs * P:(qs + 1) * P],
                            rhs=ones_sb,
                            start=False, stop=(jb == NJB - 1),
                            skip_group_check=True)

                scores(0)
                scores(1)
                for jb in range(NJB):
                    pv(jb)
                    if jb + 2 < NJB:
                        scores(jb + 2)
                recip = smallp.tile([P, 2 * NQS], f32, tag="recip", name=f"rc_{rep}_{qt}")
                nc.vector.reciprocal(out=recip, in_=psums)
                for qs in range(NQS):
                    o_sb = outp.tile([P, D], f32, tag="osb", name=f"o_{rep}_{qt}_{qs}")
                    nc.vector.tensor_scalar_mul(o_sb, po[qs], recip[:, 2 * qs:2 * qs + 1])
                    if has_bias:
                        nc.vector.tensor_add(out=o_sb, in0=o_sb, in1=bv_sb)
                    r0 = (qt * NQS + qs) * P
                    nc.scalar.dma_start(out=y[r0:r0 + P, :], in_=o_sb)

        if reps == 1:
            body(0)
        else:
            # bench-only loop. For_i carries an all-engine barrier + sem
            # reset at every back edge, so unroll U bodies per iteration:
            # the inner bodies pipeline freely (xt double-buffered) and the
            # barrier cost is paid once per U reps. Hint every engine so the
            # back-edge branch prefetches its IRAM block.
            U = 4
            assert reps % U == 0, reps
            with tc.For_i(0, reps // U, 1,
                          hint_engines=tuple(mybir.ALL_ENGINES)):
                for u in range(U):
                    body(u)
    nc.compile()
    return nc


def _prepare(x, mask, Wq, bq, Wk, bk, Wv, bv):
    """Build (or fetch cached) device program + per-core input maps."""
    x = np.asarray(x, dtype=np.float32)
    mask = np.asarray(mask)
    Wq = np.asarray(Wq, dtype=np.float32)
    Wk = np.asarray(Wk, dtype=np.float32)
    Wv = np.asarray(Wv, dtype=np.float32)
    bq = np.asarray(bq, dtype=np.float32)
    bk = np.asarray(bk, dtype=np.float32)
    bv = np.asarray(bv, dtype=np.float32)
    has_bias = bool(np.any(bq) or np.any(bk) or np.any(bv))
    has_mask = bool(np.any(mask))

    key = (has_bias, has_mask)
    if key not in _nc_cache:
        _nc_cache[key] = _build_nc(has_bias, has_mask)
    nc = _nc_cache[key]

    inv_sqrt_d = 1.0 / math.sqrt(D)
    M = (Wq.T.astype(np.float64) @ Wk.astype(np.float64)) * inv_sqrt_d
    mT_h = np.ascontiguousarray(M.astype(np.float32).astype(np.float16))
    wvT_h = np.ascontiguousarray(Wv.T.astype(np.float16))

    in_maps = []
    for c in range(NCORES):
        b, h = divmod(c, 2)
        xT_b = x[b].T.astype(np.float16)
        # rotate keys so this core's query half is columns 0..SQ-1
        xT_rot = np.ascontiguousarray(
            np.concatenate([xT_b[:, h * SQ:], xT_b[:, :h * SQ]], axis=1))
        m = {"xT": xT_rot, "mT": mT_h, "wvT": wvT_h}
        if has_bias:
            # per-key additive beta[j] = (bq Wk/sqrt(D)).x[j]; the bq.bk
            # constant shifts all keys equally and cancels in softmax.
            wt = (bq @ Wk) * inv_sqrt_d              # [D]
            m["wtl"] = np.ascontiguousarray(
                wt.reshape(D // P, P).T.astype(np.float16))
            m["bvr"] = np.ascontiguousarray(np.broadcast_to(bv, (P, D))).copy()
        if has_mask:
            keep = 1.0 - mask[b].astype(np.float32)
            keep_rot = np.concatenate([keep[h * SQ:], keep[:h * SQ]])
            m["maskf"] = np.ascontiguousarray(keep_rot.reshape(S // P, P).T)
        in_maps.append(m)
    return nc, in_maps


def _gather(res):
    out = np.empty((B, S, D), dtype=np.float32)
    for c in range(NCORES):
        b, h = divmod(c, 2)
        out[b, h * SQ:(h + 1) * SQ, :] = res.results[c]["y"]
    return out


def kernel(x, mask, Wq, bq, Wk, bk, Wv, bv):
    global last_results
    from concourse.bass_utils import run_bass_kernel_spmd

    nc, in_maps = _prepare(x, mask, Wq, bq, Wk, bk, Wv, bv)
    res = run_bass_kernel_spmd(nc, in_maps, core_ids=list(range(NCORES)))
    last_results = res
    return _gather(res)
